# revision 47
# baseline (speedup 1.0000x reference)
"""Trainium2 Bass kernel for nn_CPAMDec_Mix (dual cross-attention mix block).

Math (per batch b):
    q1 = wq1 @ x1      q2 = wq2 @ x2          (1x1 convs, [128, N] each)
    qT = concat(q1, q2) on channel -> [256, N]
    k_sT = w k_s @ y_s^T                      ([256, K])
    v_s  = y_s @ wv_s^T                       ([K, C])
    e_sT[k, n] = sum_d k_sT[d, k] qT[d, n]    ([K, N])
    attnT = softmax_k(|e1T - e2T|)            (softmax over k, no max-sub:
                                               |e| <= ~40 << 88 overflow)
    out_s = scale * (v_s^T @ attnT + bv_s) + x_s

Sharding: data-parallel over batch B=16 across 8 cores (2 batches/core),
weights replicated. Everything stays in [c, n] layout so DRAM I/O is
contiguous; softmax lives in [k, n] layout so no transposes are needed
(k-sum via ones-matmul, 1/sum broadcast comes out of the same matmul).
"""

import os
import numpy as np

import concourse.mybir as mybir
import concourse.tile as tile
from concourse import bacc
from concourse.bass import ts
from concourse.bass_utils import run_bass_kernel_spmd

F32 = mybir.dt.float32
F32R = mybir.dt.float32r
BF16 = mybir.dt.bfloat16
F8E4 = mybir.dt.float8e4
AF = mybir.ActivationFunctionType
ALU = mybir.AluOpType

# bf16 DRAM I/O for the big tensors (x, y, weights, outputs): halves HBM
# traffic; rel err contribution ~2e-3 (vs 2e-2 budget).
IOBF16 = os.environ.get("KM_IOBF16", "1") == "1"
# f32r (not f32) for attention-logit matmul operands: 4x PE rate at NT>=256
EF32R = os.environ.get("KM_EF32R", "1") == "1"
# fold k1-k2 into one kdiff tensor: halves e-matmuls, merges k-projections
KDIFF = os.environ.get("KM_KDIFF", "1") == "1"
# host-tiled x/out DRAM layout: one contiguous 8KB line per (tile,partition)
# instead of 4x 2KB lines -> 4x fewer DMA descriptors for the bulk traffic
XPACK = os.environ.get("KM_XPACK", "1") == "1"
# route this many of the 4 per-stream output drains to gpsimd (Pool)
STT_POOL = int(os.environ.get("KM_STT_POOL", "0"))
# timing-only ablations: "dma" | "noxdma" | "noodma" | "noelem"
ABL = os.environ.get("KM_ABL", "")
# store out = attn-part only (host adds the x residual): drains run on the
# Act engine (per-partition bias fold) instead of DVE stt ops
DELTA = os.environ.get("KM_DELTA", "0") == "1"
# fp8e4 storage for the out-delta (implies DELTA): quarters store traffic;
# delta sigma ~0.22 vs out ~1.07 so the quant error lands ~1e-2 rel_fro
DELTA8 = os.environ.get("KM_DELTA8", "0") == "1"
# engine per cc-chunk for the output drains (a=Act, v=DVE, p=Pool)
DRAIN = os.environ.get("KM_DRAIN", "aavv")
# software-pipelined emission: out-stage lags one tile so the PE queue is
# [q(t), ones(t-1), e(t), out(t-1)] with all deps one stage old -> no PE
# stalls, keeps the tensor engine at full p-state clock
PIPE = os.environ.get("KM_PIPE", "0") == "1"
# bf16 for ALL matmul operands (q, kdiff, expt, attnt, vv, ones): real HW
# may run bf16 at 2x the fp32r rate
MMBF16 = os.environ.get("KM_MMBF16", "0") == "1"
# stage-per-engine spreading (with PIPE): each in-order engine queue gets
# only one pipeline stage per tile, so queues never serialize across stages
QADD = os.environ.get("KM_QADD", "a")      # q bias-add engine: a|v|p
ABSENG = os.environ.get("KM_ABSENG", "a")  # |e| engine: a (Act) | v (DVE)
# pair both streams' x/out into single DRAM tensors: one load + one store
# DMA per tile instead of two of each (requires XPACK)
PAIR = os.environ.get("KM_PAIR", "0") == "1"

B, C, WH, K = 16, 512, 4096, 128
NCORES = 8
BPC = B // NCORES          # batches per core
D = 128                    # per-stream q channels (C // 4)
NT = int(os.environ.get("KM_NT", 512))   # n-tile size
NTILES = WH // NT
CCH = C // 128             # 4 c-chunks

# Matmul input dtype knobs (float32r = single-pass PE fp32, 4x faster at
# moving-dim >= 256; float32 = 2-pass full precision).
DT_Q = F32R   # q projections (also x-tile dtype)
DT_E = F32    # attention logits (kT / q tile dtype)
DT_V = F32R   # v + k projections (y / wk / wv tile dtype)
DT_O = F32R   # output projection (vv / attnt tile dtype)
DT_S = F32R   # softmax denominator ones-matmul (ones / expt tile dtype)

_PROGRAM = None
LAST_RESULTS = None


def _body(tc, io):
    nc = tc.nc
    from contextlib import ExitStack

    with ExitStack() as ctx:
        def _bufs(name, default):
            return int(os.environ.get(f"KM_BUFS_{name}", default))

        consts = ctx.enter_context(tc.tile_pool(name="consts", bufs=1))
        bpool = ctx.enter_context(tc.tile_pool(name="batch", bufs=2))
        xpool = ctx.enter_context(tc.tile_pool(name="xs", bufs=_bufs("X", 4)))
        qpool = ctx.enter_context(tc.tile_pool(name="qs", bufs=_bufs("Q", 2)))
        spool = ctx.enter_context(tc.tile_pool(name="soft", bufs=_bufs("S", 2)))
        opool = ctx.enter_context(tc.tile_pool(name="outs", bufs=_bufs("O", 3)))
        pq = ctx.enter_context(tc.tile_pool(name="pq", bufs=_bufs("PQ", 2), space="PSUM"))
        pe = ctx.enter_context(tc.tile_pool(name="pe", bufs=_bufs("PE", 2), space="PSUM"))
        psb = ctx.enter_context(tc.tile_pool(name="psb", bufs=1, space="PSUM"))
        po = ctx.enter_context(tc.tile_pool(name="po", bufs=_bufs("PO", 3), space="PSUM"))

        # dtypes for the big DRAM-backed tiles
        wq_dt = BF16 if IOBF16 else DT_Q
        wkv_dt = BF16 if IOBF16 else DT_V
        y_dt = BF16 if IOBF16 else DT_V
        x_dt = BF16 if IOBF16 else DT_Q
        o_dt = F8E4 if DELTA8 else (BF16 if IOBF16 else F32)

        def _src(ap, dt):
            # f32r tiles read f32 DRAM via bitcast; bf16 reads bf16 directly
            return ap if dt == BF16 else ap.bitcast(dt)

        # ---- constants (weights replicated per core) ----
        wq_sb, wk_sb, wv_sb, bq_sb, bk_sb, sbv_sb = {}, {}, {}, {}, {}, {}
        for s in (1, 2):
            wq_sb[s] = consts.tile([128, CCH, D], wq_dt, tag=f"wq{s}", name=f"wq{s}")
            nc.sync.dma_start(wq_sb[s][:], _src(io[f"wq{s}t"][:], wq_dt))
            wk_sb[s] = consts.tile([128, CCH, 2 * D], wkv_dt, tag=f"wk{s}", name=f"wk{s}")
            nc.sync.dma_start(wk_sb[s][:], _src(io[f"wk{s}t"][:], wkv_dt))
            wv_sb[s] = consts.tile([128, CCH, C], wkv_dt, tag=f"wv{s}", name=f"wv{s}")
            nc.sync.dma_start(wv_sb[s][:], _src(io[f"wv{s}t"][:], wkv_dt))
            bq_sb[s] = consts.tile([128, 1], F32, tag=f"bq{s}", name=f"bq{s}")
            nc.sync.dma_start(bq_sb[s][:], io[f"bq{s}"][:])
            bk_sb[s] = consts.tile([128, 2], F32, tag=f"bk{s}", name=f"bk{s}")
            nc.sync.dma_start(bk_sb[s][:], io[f"bk{s}"][:])
            sbv_sb[s] = consts.tile([128, CCH], F32, tag=f"sbv{s}", name=f"sbv{s}")
            nc.sync.dma_start(sbv_sb[s][:], io[f"sbv{s}"][:])
        scale_sb = consts.tile([128, 1], F32, tag="scale")
        nc.sync.dma_start(scale_sb[:], io["scale_rep"][:])
        if MMBF16:
            ones_sb = consts.tile([128, 128], BF16, tag="ones")
            nc.vector.memset(ones_sb[:], 1.0)
        else:
            ones_sb = consts.tile([128, 128], DT_S, tag="ones")
            nc.sync.dma_start(ones_sb[:], io["ones"][:].bitcast(DT_S))
        dummy_ot = None
        if ABL in ("dma", "noelem"):
            dummy_ot = consts.tile([128, CCH, NT], o_dt, tag="dummy")
            nc.vector.memset(dummy_ot[:], 0.0)

        e_dt = BF16 if MMBF16 else (F32R if EF32R else DT_E)
        o_mm_dt = BF16 if MMBF16 else DT_O
        s_mm_dt = BF16 if MMBF16 else DT_S

        # ---- software-pipeline stages (PIPE=1): out-stage lags one tile ----
        pend = [None]

        def _pf_front(b, nt, x_ap, o_ap, kd, vv):
            """x loads + q matmuls + q bias-act for tile nt; returns state."""
            st = {"nt": nt, "o_ap": o_ap, "vv": vv, "kd": kd}
            xt = {}
            for s in (1, 2):
                xt[s] = xpool.tile([128, CCH, NT], x_dt, tag=f"x{s}", name=f"x{s}")
                xsrc = x_ap[s][nt] if XPACK else x_ap[s][:, :, ts(nt, NT)]
                nc.sync.dma_start(xt[s][:], _src(xsrc, x_dt))
            st["xt"] = xt
            q = {}
            for s in (1, 2):
                pqt = pq.tile([128, NT], F32, tag="pq", name="pqt")
                for cc in range(CCH):
                    nc.tensor.matmul(
                        pqt[:], wq_sb[s][:, cc, :], xt[s][:, cc, :],
                        start=(cc == 0), stop=(cc == CCH - 1),
                    )
                q[s] = qpool.tile([128, NT], e_dt, tag=f"q{s}", name=f"q{s}")
                if QADD == "a":
                    nc.scalar.activation(q[s][:], pqt[:], AF.Identity,
                                         bias=bq_sb[s][:])
                else:
                    qeng = nc.vector if QADD == "v" else nc.gpsimd
                    qeng.tensor_scalar_add(q[s][:], pqt[:], bq_sb[s][:])
            st["q"] = q
            return st

        def _pf_e(st):
            pdiff = pe.tile([128, NT], F32, tag="pe1", name="pdiff")
            for dc in range(2):
                nc.tensor.matmul(
                    pdiff[:], st["kd"][:, dc, :], st["q"][dc + 1][:],
                    start=(dc == 0), stop=(dc == 1),
                )
            st["pdiff"] = pdiff

        def _pf_absexp(st):
            adiff = spool.tile([128, NT], F32, tag="adiff")
            if ABSENG == "v":
                # |e| = max(-e, e) on DVE, freeing the Act queue for exp only
                nc.vector.scalar_tensor_tensor(
                    adiff[:], st["pdiff"], -1.0, st["pdiff"],
                    op0=ALU.mult, op1=ALU.max,
                )
            else:
                nc.scalar.activation(adiff[:], st["pdiff"], AF.Abs)
            expt = spool.tile([128, NT], s_mm_dt, tag="expt")
            nc.scalar.activation(expt[:], adiff[:], AF.Exp)
            st["expt"] = expt

        def _pf_ones(st):
            psum_s = psb.tile([128, NT], F32, tag="psb", name="psum_s")
            nc.tensor.matmul(psum_s[:], ones_sb[:], st["expt"][:])
            st["psum_s"] = psum_s

        def _pf_norm(st):
            rb = spool.tile([128, NT], F32, tag="rb")
            nc.vector.reciprocal(rb[:], st["psum_s"][:])
            attnt = spool.tile([128, NT], o_mm_dt, tag="attnt")
            nc.vector.tensor_mul(attnt[:], st["expt"][:], rb[:])
            st["attnt"] = attnt

        def _pf_out(st):
            nt, o_ap, vv, xt = st["nt"], st["o_ap"], st["vv"], st["xt"]
            for s in (1, 2):
                ot = opool.tile([128, CCH, NT], o_dt, tag=f"o{s}", name=f"o{s}")
                for cc in range(CCH):
                    pot = po.tile([128, NT], F32, tag="po", name="pot")
                    nc.tensor.matmul(pot[:], vv[s][:, ts(cc, 128)], st["attnt"][:])
                    if DELTA:
                        deng = {"a": nc.scalar, "v": nc.vector,
                                "p": nc.gpsimd}[DRAIN[cc]]
                        if deng is nc.scalar:
                            deng.activation(
                                ot[:, cc, :], pot[:], AF.Identity,
                                bias=sbv_sb[s][:, cc : cc + 1],
                            )
                        else:
                            deng.tensor_scalar_add(
                                ot[:, cc, :], pot[:], sbv_sb[s][:, cc : cc + 1],
                            )
                    else:
                        seng = nc.gpsimd if cc < STT_POOL else nc.vector
                        seng.scalar_tensor_tensor(
                            ot[:, cc, :], pot[:], sbv_sb[s][:, cc : cc + 1],
                            xt[s][:, cc, :], op0=ALU.add, op1=ALU.add,
                        )
                oeng = nc.scalar if os.environ.get("KM_RING", "1") in ("1", "3") else nc.sync
                odst = o_ap[s][nt] if XPACK else o_ap[s][:, :, ts(nt, NT)]
                oeng.dma_start(odst, ot[:])

        for _rep in range(int(os.environ.get("KM_REPEAT", 1))):
         for b in range(BPC):
            # ---- per-batch: k_sT [128, 2, 128] and v_s [128, C] ----
            kT, vv, ydict = {}, {}, {}
            for s in (1, 2):
                ydict[s] = bpool.tile([128, CCH, K], y_dt, tag=f"y{s}", name=f"y{s}")
                nc.sync.dma_start(ydict[s][:], _src(io[f"y{s}t"][b], y_dt))
            if KDIFF:
                # kdiff^T = wk1@y1^T - wk2@y2^T (wk2, bk2 host-negated),
                # accumulated across both streams in one PSUM group
                kd = bpool.tile([128, 2, K], e_dt, tag="kd", name="kd")
                for dc in range(2):
                    pk = pe.tile([128, NT], F32, tag="pe1", name="pk")[:, :K]
                    for s in (1, 2):
                        for cc in range(CCH):
                            nc.tensor.matmul(
                                pk[:],
                                wk_sb[s][:, cc, ts(dc, D)],
                                ydict[s][:, cc, :],
                                start=(s == 1 and cc == 0),
                                stop=(s == 2 and cc == CCH - 1),
                            )
                    nc.scalar.activation(
                        kd[:, dc, :], pk[:], AF.Identity,
                        bias=bk_sb[1][:, dc : dc + 1],
                    )
            for s in (1, 2):
                yt = ydict[s]
                if not KDIFF:
                    kT[s] = bpool.tile([128, 2, K], DT_E, tag=f"k{s}", name=f"k{s}")
                    for dc in range(2):
                        pk = pe.tile([128, NT], F32, tag="pe1", name="pk")[:, :K]
                        for cc in range(CCH):
                            nc.tensor.matmul(
                                pk[:],
                                wk_sb[s][:, cc, ts(dc, D)],
                                yt[:, cc, :],
                                start=(cc == 0),
                                stop=(cc == CCH - 1),
                            )
                        # stream 2 is negated (and its bias host-negated) so
                        # the e-matmuls can accumulate e1 - e2 in one PSUM bank
                        nc.scalar.activation(
                            kT[s][:, dc, :], pk[:], AF.Identity,
                            bias=bk_sb[s][:, dc : dc + 1],
                            scale=(1.0 if s == 1 else -1.0),
                        )
                vv[s] = bpool.tile([128, C], o_mm_dt, tag=f"v{s}", name=f"v{s}")
                pv = po.tile([128, C], F32, tag="po", name="pv")
                for cc in range(CCH):
                    nc.tensor.matmul(
                        pv[:],
                        yt[:, cc, :],
                        wv_sb[s][:, cc, :],
                        start=(cc == 0),
                        stop=(cc == CCH - 1),
                    )
                # v_s scaled by `scale` here; bias bv folded into the output
                # residual (attn rows sum to 1).
                nc.scalar.activation(
                    vv[s][:], pv[:], AF.Copy, bias=0.0, scale=scale_sb[:],
                )

            if PAIR:
                xp_ap = io["x12"][b]
                op_ap = io["out12"][b]
                x_ap = o_ap = None
            elif XPACK:
                x_ap = {s: io[f"x{s}"][b] for s in (1, 2)}
                o_ap = {s: io[f"out{s}"][b] for s in (1, 2)}
            else:
                x_ap = {s: io[f"x{s}"][b].rearrange("(co p) n -> p co n", p=128)
                        for s in (1, 2)}
                o_ap = {s: io[f"out{s}"][b].rearrange("(co p) n -> p co n", p=128)
                        for s in (1, 2)}

            if PIPE:
                assert KDIFF, "PIPE requires KDIFF"
                for nt in range(int(os.environ.get("KM_NTILES", NTILES))):
                    st = _pf_front(b, nt, x_ap, o_ap, kd, vv)
                    if pend[0] is not None:
                        _pf_ones(pend[0])
                    _pf_e(st)
                    if pend[0] is not None:
                        _pf_norm(pend[0])
                        _pf_out(pend[0])
                    _pf_absexp(st)
                    pend[0] = st
                continue

            if ABL == "noxdma":
                xhold = {}
                for s in (1, 2):
                    xhold[s] = bpool.tile([128, CCH, NT], x_dt, tag=f"xh{s}", name=f"xh{s}")
                    src0 = x_ap[s][0] if XPACK else x_ap[s][:, :, ts(0, NT)]
                    nc.sync.dma_start(xhold[s][:], _src(src0, x_dt))

            for nt in range(int(os.environ.get("KM_NTILES", NTILES))):
                nsl = ts(nt, NT)
                if ABL == "dma":
                    for s in (1, 2):
                        xt_d = xpool.tile([128, CCH, NT], x_dt, tag=f"x{s}", name=f"x{s}")
                        xsrc = x_ap[s][nt] if XPACK else x_ap[s][:, :, nsl]
                        nc.sync.dma_start(xt_d[:], _src(xsrc, x_dt))
                        odst = o_ap[s][nt] if XPACK else o_ap[s][:, :, nsl]
                        nc.scalar.dma_start(odst, dummy_ot[:])
                    continue
                # ---- load x tiles ----
                xt = {}
                if ABL == "noxdma":
                    xt = xhold
                elif PAIR:
                    xt12 = xpool.tile([128, 2, CCH, NT], x_dt, tag="x12", name="x12")
                    nc.sync.dma_start(xt12[:], _src(xp_ap[nt], x_dt))
                    xt = {1: xt12[:, 0], 2: xt12[:, 1]}
                else:
                    for s in (1, 2):
                        xt[s] = xpool.tile([128, CCH, NT], x_dt, tag=f"x{s}", name=f"x{s}")
                        xeng = nc.scalar if (s == 2 and os.environ.get("KM_RING", "1") in ("2", "3")) else nc.sync
                        xsrc = x_ap[s][nt] if XPACK else x_ap[s][:, :, nsl]
                        xeng.dma_start(xt[s][:], _src(xsrc, x_dt))

                # ---- q projections: qT chunk s -> [128, NT] ----
                q = {}
                for s in (1, 2):
                    pqt = pq.tile([128, NT], F32, tag="pq", name="pqt")
                    for cc in range(CCH):
                        nc.tensor.matmul(
                            pqt[:],
                            wq_sb[s][:, cc, :],
                            xt[s][:, cc, :],
                            start=(cc == 0),
                            stop=(cc == CCH - 1),
                        )
                    q[s] = qpool.tile([128, NT], e_dt, tag=f"q{s}", name=f"q{s}")
                    nc.scalar.activation(
                        q[s][:], pqt[:], AF.Identity, bias=bq_sb[s][:],
                    )

                # ---- attention logit diff e1T - e2T, accumulated in PSUM ----
                pdiff = pe.tile([128, NT], F32, tag="pe1", name="pdiff")
                if KDIFF:
                    for dc in range(2):
                        nc.tensor.matmul(
                            pdiff[:],
                            kd[:, dc, :],
                            q[dc + 1][:],
                            start=(dc == 0),
                            stop=(dc == 1),
                        )
                else:
                    # kT[2] is pre-negated, so all 4 matmuls add into one bank
                    for s in (1, 2):
                        for dc in range(2):
                            nc.tensor.matmul(
                                pdiff[:],
                                kT[s][:, dc, :],
                                q[dc + 1][:],
                                start=(s == 1 and dc == 0),
                                stop=(s == 2 and dc == 1),
                            )

                # ---- softmax over k (partition dim), no max subtraction ----
                attnt = spool.tile([128, NT], o_mm_dt, tag="attnt")
                if ABL != "noelem":
                    adiff = spool.tile([128, NT], F32, tag="adiff")
                    nc.scalar.activation(adiff[:], pdiff[:], AF.Abs)
                    expt = spool.tile([128, NT], s_mm_dt, tag="expt")
                    nc.scalar.activation(expt[:], adiff[:], AF.Exp)
                    # all-partition sum broadcast: ones[128,128]^T @ expt
                    psum_s = psb.tile([128, NT], F32, tag="psb", name="psum_s")
                    nc.tensor.matmul(
                        psum_s[:], ones_sb[:], expt[:],
                    )
                    rb = spool.tile([128, NT], F32, tag="rb")
                    nc.vector.reciprocal(rb[:], psum_s[:])
                    nc.vector.tensor_mul(attnt[:], expt[:], rb[:])
                elif nt < 2 and b == 0 and _rep == 0:
                    # init both rotating pool buffers once (timing ablation)
                    nc.vector.memset(attnt[:], 0.5)

                # ---- outputs: out_s[c, n] = v_s^T @ attnT + s*bv_s (+ x_s) ----
                ot12 = (opool.tile([128, 2, CCH, NT], o_dt, tag="o12", name="o12")
                        if PAIR else None)
                for s in (1, 2):
                    ot = ot12[:, s - 1] if PAIR else \
                        opool.tile([128, CCH, NT], o_dt, tag=f"o{s}", name=f"o{s}")
                    for cc in range(CCH):
                        pot = po.tile([128, NT], F32, tag="po", name="pot")
                        nc.tensor.matmul(
                            pot[:],
                            vv[s][:, ts(cc, 128)],
                            attnt[:],
                        )
                        if ABL == "noelem":
                            continue
                        if DELTA:
                            deng = {"a": nc.scalar, "v": nc.vector,
                                    "p": nc.gpsimd}[DRAIN[cc]]
                            if deng is nc.scalar:
                                deng.activation(
                                    ot[:, cc, :], pot[:], AF.Identity,
                                    bias=sbv_sb[s][:, cc : cc + 1],
                                )
                            else:
                                deng.tensor_scalar_add(
                                    ot[:, cc, :], pot[:],
                                    sbv_sb[s][:, cc : cc + 1],
                                )
                        else:
                            seng = nc.gpsimd if cc < STT_POOL else nc.vector
                            seng.scalar_tensor_tensor(
                                ot[:, cc, :],
                                pot[:],
                                sbv_sb[s][:, cc : cc + 1],
                                xt[s][:, cc, :],
                                op0=ALU.add,
                                op1=ALU.add,
                            )
                    if ABL == "noodma":
                        continue
                    oeng = nc.scalar if os.environ.get("KM_RING", "1") in ("1", "3") else nc.sync
                    if PAIR:
                        if s == 2:
                            oeng.dma_start(op_ap[nt], ot12[:])
                        continue
                    odst = o_ap[s][nt] if XPACK else o_ap[s][:, :, nsl]
                    oeng.dma_start(odst, ot[:] if ABL != "noelem" else dummy_ot[:])

        if PIPE and pend[0] is not None:
            _pf_ones(pend[0])
            _pf_norm(pend[0])
            _pf_out(pend[0])


def build_program():
    nc = bacc.Bacc(
        "TRN2", target_bir_lowering=False, debug=False, enable_asserts=False,
    )
    io = {}
    big = BF16 if IOBF16 else F32

    def din(name, shape, dt=F32):
        io[name] = nc.dram_tensor(name, shape, dt, kind="ExternalInput").ap()

    def dout(name, shape, dt=F32):
        io[name] = nc.dram_tensor(name, shape, dt, kind="ExternalOutput").ap()

    if PAIR:
        din("x12", [BPC, NTILES, 128, 2, CCH, NT], big)
    else:
        xshape = [BPC, NTILES, 128, CCH, NT] if XPACK else [BPC, C, WH]
        din("x1", xshape, big)
        din("x2", xshape, big)
    din("y1t", [BPC, 128, CCH, K], big)
    din("y2t", [BPC, 128, CCH, K], big)
    for s in (1, 2):
        din(f"wq{s}t", [128, CCH, D], big)
        din(f"wk{s}t", [128, CCH, 2 * D], big)
        din(f"wv{s}t", [128, CCH, C], big)
        din(f"bq{s}", [128, 1])
        din(f"bk{s}", [128, 2])
        din(f"sbv{s}", [128, CCH])
    din("scale_rep", [128, 1])
    din("ones", [128, 128])
    odt = F8E4 if DELTA8 else big
    if PAIR:
        dout("out12", [BPC, NTILES, 128, 2, CCH, NT], odt)
    else:
        oshape = [BPC, NTILES, 128, CCH, NT] if XPACK else [BPC, C, WH]
        dout("out1", oshape, odt)
        dout("out2", oshape, odt)

    with tile.TileContext(nc) as tc:
        _body(tc, io)
    nc.compile()
    return nc


def _get_program():
    global _PROGRAM
    if _PROGRAM is None:
        _PROGRAM = build_program()
    return _PROGRAM


def _to_chunked(w):
    # host weight [out, in] -> transposed chunked SBUF layout [p, co, out]
    # (wT[c, out] with input-channel c = co*128 + p), contiguous for DMA
    out_dim, in_dim = w.shape
    return np.ascontiguousarray(
        w.T.reshape(in_dim // 128, 128, out_dim).transpose(1, 0, 2)
    )


def _bias_chunks(bv):
    # [d] -> [128, d//128] with d = dc*128 + p
    return np.ascontiguousarray(bv.reshape(-1, 128).T)


def prepare_in_maps(inputs):
    f = lambda a: np.ascontiguousarray(np.asarray(a, dtype=np.float32))
    if IOBF16:
        import ml_dtypes
        big = lambda a: np.ascontiguousarray(a.astype(ml_dtypes.bfloat16))
    else:
        big = lambda a: a
    def xpk(x):
        # [B, C, WH] -> [B, NTILES, 128, CCH, NT]: per-(tile,partition)
        # contiguous lines of CCH*NT elements
        return np.ascontiguousarray(
            x.reshape(B, CCH, 128, NTILES, NT).transpose(0, 3, 2, 1, 4)
        )

    x1 = f(inputs["x1"]).reshape(B, C, WH)
    x2 = f(inputs["x2"]).reshape(B, C, WH)
    if XPACK:
        x1, x2 = xpk(x1), xpk(x2)
    if PAIR:
        x12 = np.ascontiguousarray(np.stack([x1, x2], axis=3))
    # y^T per batch in chunked layout [b, p, co, k]
    def yt_chunk(y):
        ytr = f(y).transpose(0, 2, 1)  # [B, C, K]
        return np.ascontiguousarray(
            ytr.reshape(B, CCH, 128, K).transpose(0, 2, 1, 3)
        )
    y1t = yt_chunk(inputs["y1"])
    y2t = yt_chunk(inputs["y2"])
    scale = float(np.asarray(inputs["scale"]).reshape(-1)[0])

    shared = {"scale_rep": np.full((128, 1), scale, np.float32),
              "ones": np.ones((128, 128), np.float32)}
    for s in (1, 2):
        wk_sign = 1.0 if (s == 1 or not KDIFF) else -1.0
        shared[f"wq{s}t"] = big(_to_chunked(f(inputs[f"wq{s}"])))
        shared[f"wk{s}t"] = big(_to_chunked(wk_sign * f(inputs[f"wk{s}"])))
        shared[f"wv{s}t"] = big(_to_chunked(f(inputs[f"wv{s}"])))
        shared[f"bq{s}"] = f(inputs[f"bq{s}"]).reshape(128, 1)
        bk_sign = 1.0 if s == 1 else -1.0
        shared[f"bk{s}"] = _bias_chunks(bk_sign * f(inputs[f"bk{s}"]))
        shared[f"sbv{s}"] = _bias_chunks(scale * f(inputs[f"bv{s}"]))
    if KDIFF:
        # kdiff bias = bk1 - bk2, loaded via the bk1 slot
        shared["bk1"] = _bias_chunks(f(inputs["bk1"]) - f(inputs["bk2"]))

    in_maps = []
    for c in range(NCORES):
        sl = slice(BPC * c, BPC * (c + 1))
        xm = ({"x12": big(np.ascontiguousarray(x12[sl]))} if PAIR else
              {"x1": big(np.ascontiguousarray(x1[sl])),
               "x2": big(np.ascontiguousarray(x2[sl]))})
        in_maps.append({
            **xm,
            "y1t": big(np.ascontiguousarray(y1t[sl])),
            "y2t": big(np.ascontiguousarray(y2t[sl])),
            **shared,
        })
    return in_maps


def kernel(**inputs):
    global LAST_RESULTS
    nc = _get_program()
    in_maps = prepare_in_maps(inputs)
    try:
        res = run_bass_kernel_spmd(nc, in_maps, list(range(NCORES)))
    except Exception:
        # transient NRT device hiccups have been observed; retry once
        res = run_bass_kernel_spmd(nc, in_maps, list(range(NCORES)))
    LAST_RESULTS = res

    def unx(o):
        # [B, NTILES, 128, CCH, NT] -> [B, C, WH]
        return o.transpose(0, 3, 2, 1, 4).reshape(B, C, WH)

    def unpack(name):
        o = np.concatenate(
            [np.asarray(res.results[c][name]).astype(np.float32)
             for c in range(NCORES)], axis=0)
        if XPACK:
            o = unx(o)
        return o.reshape(B, C, 64, 64)

    if PAIR:
        o12 = np.concatenate(
            [np.asarray(res.results[c]["out12"]).astype(np.float32)
             for c in range(NCORES)], axis=0)
        out1 = unx(o12[:, :, :, 0]).reshape(B, C, 64, 64)
        out2 = unx(o12[:, :, :, 1]).reshape(B, C, 64, 64)
    else:
        out1, out2 = unpack("out1"), unpack("out2")
    if DELTA:
        out1 = out1 + np.asarray(inputs["x1"], dtype=np.float32)
        out2 = out2 + np.asarray(inputs["x2"], dtype=np.float32)
    return out1, out2


def bench(inputs, iters=30, repeat=1, nc=None):
    """Time warm back-to-back executions of the compiled NEFF on 8 cores.

    Replicates run_bass_via_pjrt's shard_map jit, but without output-buffer
    donation so device-resident inputs can be reused across calls (this
    kernel writes every output element, so uninitialized result buffers are
    fine). Returns (per_call_seconds, results_list).
    """
    import time as _time
    import jax
    import concourse.mybir as _mybir
    from jax.experimental.shard_map import shard_map
    from jax.sharding import Mesh, PartitionSpec
    from concourse.bass2jax import _bass_exec_p, install_neuronx_cc_hook

    from concourse.bass2jax import partition_id_tensor
    install_neuronx_cc_hook()
    if nc is None:
        nc = _get_program()
    in_maps = prepare_in_maps(inputs)

    partition_name = nc.partition_id_tensor.name if nc.partition_id_tensor else None
    in_names, out_names, out_avals = [], [], []
    for alloc in nc.m.functions[0].allocations:
        if not isinstance(alloc, _mybir.MemoryLocationSet):
            continue
        name = alloc.memorylocations[0].name
        if alloc.kind == "ExternalInput":
            if name != partition_name:
                in_names.append(name)
        elif alloc.kind == "ExternalOutput":
            out_names.append(name)
            out_avals.append(jax.core.ShapedArray(
                tuple(alloc.tensor_shape), _mybir.dt.np(alloc.dtype)))
    n_params = len(in_names)
    all_names = in_names + out_names
    if partition_name is not None:
        all_names = all_names + [partition_name]

    def _call(ins, bufs):
        operands = list(ins) + list(bufs)
        if partition_name is not None:
            operands.append(partition_id_tensor())
        return tuple(_bass_exec_p.bind(
            *operands,
            out_avals=tuple(out_avals),
            in_names=tuple(all_names),
            out_names=tuple(out_names),
            lowering_input_output_aliases=(),
            sim_require_finite=True,
            sim_require_nnan=True,
            nc=nc,
        ))

    def _body(*args):
        ins, bufs = args[:n_params], args[n_params:]
        out = _call(ins, bufs)
        for _ in range(repeat - 1):
            # chain on previous outputs: serializes executions on-device so
            # one host dispatch amortizes over `repeat` NEFF runs
            out = _call(ins, out)
        return out

    devices = jax.devices()[:NCORES]
    mesh = Mesh(np.asarray(devices), ("core",))
    nin = n_params + len(out_names)
    f = jax.jit(
        shard_map(
            _body, mesh=mesh,
            in_specs=(PartitionSpec("core"),) * nin,
            out_specs=(PartitionSpec("core"),) * len(out_names),
            check_rep=False,
        ),
        keep_unused=True,
    )
    sharding = jax.sharding.NamedSharding(mesh, PartitionSpec("core"))
    concat_in = [
        jax.device_put(
            np.concatenate([np.asarray(in_maps[c][nm]) for c in range(NCORES)], axis=0),
            sharding)
        for nm in in_names
    ]
    concat_zeros = [
        jax.device_put(
            np.zeros((NCORES * av.shape[0], *av.shape[1:]), av.dtype), sharding)
        for av in out_avals
    ]
    args = concat_in + concat_zeros

    out = f(*args)
    jax.block_until_ready(out)
    t0 = _time.perf_counter()
    for _ in range(iters):
        out = f(*args)
    jax.block_until_ready(out)
    dt = (_time.perf_counter() - t0) / iters
    results = [
        {nm: np.asarray(out[i]).reshape(NCORES, *out_avals[i].shape)[c]
         for i, nm in enumerate(out_names)}
        for c in range(NCORES)
    ]
    return dt, results

